# revision 8
# baseline (speedup 1.0000x reference)
"""Cayley orthogonal transform kernel for Trainium2 (8 NeuronCores).

Math: per head h, y = (I - S) ((1+eps) I + S)^{-1} x applied along D=128,
where S = S_raw - S_raw^T is skew-symmetric.

Strategy:
  * Host: skew-symmetrize S_raw, and lay x out as xT[h, d, token] (token-major
    per head) so the device only ever runs plain matmuls - no on-device
    transposes.  Heads are sharded 2-per-core across the 8 cores (tensor
    parallel, embarrassingly parallel per the problem structure).
  * Device (per core): build W^T = ((1+eps)I - S)^{-1} (I + S) per head with a
    Newton-Schulz iteration in fp16 (converges to ~5e-4 in 5 iterations since
    ||S||_2 ~ 1.6; fp16 matmuls run the PE 4x faster than fp32 so the whole
    inverse costs ~8us, hidden under the x DMA stream).  Then stream the
    (128 x 16384) token panels through the PE in 512-column fp16 matmuls:
        yT[h] = W @ xT[h]
    All of x (8 MiB fp16) is loaded into SBUF up-front with large DMAs that
    saturate the 16 DMA queues from t=0; PSUM is evacuated to fp16 SBUF
    alternating Vector/Scalar engines and finished 1 MiB output tiles are
    DMA'd back.  Everything is fp16 over the wire (x in, y out, W in the PE):
    end-to-end rel_l2 ~ 4e-4, and the kernel runs at the 2-byte HBM roofline
    (~17 MB of DRAM traffic per core; DMA ~80% of the kernel span).
  * Host: widen y to fp32 and inverse layout transform back to (B, H, N, D).
"""

import os
import sys

import numpy as np

B, H, N, D = 4, 16, 4096, 128
N_CORES = 8
HPC = H // N_CORES          # heads per core
T = B * N                   # tokens per head
MM = 512                    # columns per matmul (one PSUM bank)
# x tile sizes per head: first tile engages all 16 DMA queues; large tiles
# after to amortize DMA trigger/semaphore overhead.
XTILES = {0: (4096, 4096, 8192), 1: (8192, 8192)}
OUT_CHUNK = 4096            # columns per output store (1 MiB fp16)
NS_ITERS = 5                # Newton-Schulz iterations
NS_C = 0.42                 # NS initial scale: X0 = c * G^T  (safe for ||S||<~1.9)
EPS = 1e-5

_CACHE = {}


def _ensure_path():
    for p in ("/opt/trn_rl_repo", "/root/.axon_site/_ro/trn_rl_repo"):
        if os.path.isdir(p) and p not in sys.path:
            sys.path.insert(0, p)
    _install_ntff_hook()


def _install_ntff_hook():
    """The agent image's ``antenv`` lacks ``axon_hooks``, which makes
    ``run_bass_kernel_spmd(trace=True)`` crash instead of degrading.  Provide
    the module and register the ctypes NTFF hook the boot shim would have."""
    if "antenv.axon_hooks" in sys.modules:
        return
    try:
        import types

        import antenv

        if hasattr(antenv, "axon_hooks"):
            return
        mod = types.ModuleType("antenv.axon_hooks")
        state = {"hook": None}
        mod.set_axon_ntff_profile_hook = lambda h: state.__setitem__("hook", h)
        mod.get_axon_ntff_profile_hook = lambda: state["hook"]
        sys.modules["antenv.axon_hooks"] = mod
        antenv.axon_hooks = mod
        try:
            from trn_agent_boot.trn_boot import _ntff_profile_via_ctypes

            so_path = "/opt/axon/libaxon_pjrt.so"
            if os.path.exists(so_path):
                mod.set_axon_ntff_profile_hook(_ntff_profile_via_ctypes(so_path))
        except Exception:
            pass  # hook stays None -> concourse logs + skips tracing
    except Exception:
        pass


def _build_nc():
    """Build the (single-program SPMD) Bass kernel for one core's shard."""
    _ensure_path()
    import concourse.tile as tile
    from concourse import bacc, mybir
    from concourse.masks import make_identity

    f16 = mybir.dt.float16
    f32 = mybir.dt.float32
    Alu = mybir.AluOpType

    nc = bacc.Bacc("TRN2", target_bir_lowering=False, debug=False)
    x_d = nc.dram_tensor("xh", [HPC * D, T], f16, kind="ExternalInput").ap()
    s_d = nc.dram_tensor("s", [HPC * D, D], f32, kind="ExternalInput").ap()
    yT_d = nc.dram_tensor("yT", [HPC * D, T], f16, kind="ExternalOutput").ap()

    with tile.TileContext(nc) as tc:
        with (
            tc.tile_pool(name="const", bufs=1) as const_pool,
            tc.tile_pool(name="ns", bufs=2) as ns_pool,
            tc.tile_pool(name="xin", bufs=1) as in_pool,
            tc.tile_pool(name="yout", bufs=1) as out_pool,
            tc.tile_pool(name="mmps", bufs=8, space="PSUM") as ps_pool,
        ):
            # --- DMAs first: tiny S, then all of x (stays resident in SBUF).
            s_sbs = []
            for h in range(HPC):
                s_sb = const_pool.tile([D, D], f32, tag=f"s{h}")
                nc.sync.dma_start(out=s_sb, in_=s_d[h * D:(h + 1) * D, :])
                s_sbs.append(s_sb)
            xts = {0: [], 1: []}
            for h in range(HPC):
                c0 = 0
                for sz in XTILES[h]:
                    xt = in_pool.tile([D, sz], f16, name=f"x{h}_{c0}",
                                      tag=f"x{h}_{c0}")
                    nc.sync.dma_start(
                        out=xt, in_=x_d[h * D:(h + 1) * D, c0:c0 + sz])
                    xts[h].append((c0, xt))
                    c0 += sz

            # --- Newton-Schulz per head (fp16): W^T = G^{-1} A, G = (1+eps)I-S,
            # A = G^T = (1+eps)I + S.  (W^T = G^{-1}(I+S) = G^{-1}A - eps G^{-1};
            # the eps term is ~1e-5, far below fp16 resolution.)
            # bass matmul computes lhsT.T @ rhs, so GX = matmul(lhsT=A, rhs=X);
            # the XT chain tracks X^T to avoid on-device transposes.
            ident = const_pool.tile([D, D], f32, tag="ident")
            make_identity(nc, ident)
            twoE = const_pool.tile([D, D], f32, tag="twoE")
            nc.vector.tensor_scalar_mul(twoE, ident, 2.0)

            a16s, Xs, XTs = [], [], []
            for h in range(HPC):
                a16 = const_pool.tile([D, D], f16, tag=f"a{h}")
                nc.vector.scalar_tensor_tensor(
                    out=a16, in0=ident, scalar=1.0 + EPS, in1=s_sbs[h],
                    op0=Alu.mult, op1=Alu.add)
                g16 = const_pool.tile([D, D], f16, tag=f"g{h}")
                nc.vector.scalar_tensor_tensor(
                    out=g16, in0=ident, scalar=1.0 + EPS, in1=s_sbs[h],
                    op0=Alu.mult, op1=Alu.subtract)
                X = ns_pool.tile([D, D], f16, tag=f"x{h}")
                nc.vector.tensor_scalar_mul(X, a16, NS_C)    # X0 = c G^T
                XT = ns_pool.tile([D, D], f16, tag=f"xt{h}")
                nc.vector.tensor_scalar_mul(XT, g16, NS_C)   # X0^T = c G
                a16s.append(a16)
                Xs.append(X)
                XTs.append(XT)

            # Interleave the two heads' chains so each head's vector-op
            # latency hides under the other head's matmuls.
            for k in range(NS_ITERS):
                t2s = []
                for h in range(HPC):
                    t_ps = ps_pool.tile([D, D], f32, tag="mm", name="t_ps")
                    nc.tensor.matmul(t_ps, lhsT=a16s[h], rhs=Xs[h],
                                     start=True, stop=True)       # G X
                    t2 = ns_pool.tile([D, D], f16, tag=f"t2{h}")
                    nc.vector.tensor_sub(t2, twoE, t_ps)          # 2I - G X
                    t2s.append(t2)
                for h in range(HPC):
                    if k < NS_ITERS - 1:
                        xn_ps = ps_pool.tile([D, D], f32, tag="mm", name="xn_ps")
                        nc.tensor.matmul(xn_ps, lhsT=XTs[h], rhs=t2s[h],
                                         start=True, stop=True)   # X T2
                        Xn = ns_pool.tile([D, D], f16, tag=f"x{h}")
                        nc.scalar.copy(Xn, xn_ps)
                        Xs[h] = Xn
                    xtn_ps = ps_pool.tile([D, D], f32, tag="mm", name="xtn_ps")
                    nc.tensor.matmul(xtn_ps, lhsT=t2s[h], rhs=XTs[h],
                                     start=True, stop=True)       # (X T2)^T
                    XTn = ns_pool.tile([D, D], f16, tag=f"xt{h}")
                    nc.scalar.copy(XTn, xtn_ps)
                    XTs[h] = XTn

            w16s = []
            for h in range(HPC):
                wt_ps = ps_pool.tile([D, D], f32, tag="mm", name="wt_ps")
                nc.tensor.matmul(wt_ps, lhsT=XTs[h], rhs=a16s[h],
                                 start=True, stop=True)           # G^{-1} A
                w16 = const_pool.tile([D, D], f16, tag=f"w{h}")
                nc.vector.tensor_copy(w16, wt_ps)
                w16s.append(w16)

            # --- streaming panel matmul: yT[h] = W @ xT[h], fp16
            for h in range(HPC):
                yts = {}
                for c0, xt in xts[h]:
                    sz = xt.shape[-1]
                    for j in range(sz // MM):
                        col = c0 + j * MM          # absolute column in head
                        oc, oj = divmod(col, OUT_CHUNK)
                        if oj == 0:
                            yts[oc] = out_pool.tile([D, OUT_CHUNK], f16,
                                                    name=f"y{h}_{oc}",
                                                    tag=f"y{h}_{oc}")
                        ps = ps_pool.tile([D, MM], f32, tag="mm", name="ps")
                        nc.tensor.matmul(ps, lhsT=w16s[h],
                                         rhs=xt[:, j * MM:(j + 1) * MM],
                                         start=True, stop=True)
                        dst = yts[oc][:, oj:oj + MM]
                        if (col // MM) % 2 == 0:
                            nc.vector.tensor_copy(dst, ps)
                        else:
                            nc.scalar.copy(dst, ps)
                        if oj + MM == OUT_CHUNK:
                            nc.scalar.dma_start(
                                out=yT_d[h * D:(h + 1) * D,
                                         oc * OUT_CHUNK:(oc + 1) * OUT_CHUNK],
                                in_=yts[oc])
    nc.compile()
    return nc


def _get_nc():
    if "nc" not in _CACHE:
        _CACHE["nc"] = _build_nc()
    return _CACHE["nc"]


def _prep_inputs(x, S_raw):
    """Host-side shard + layout prep. Returns per-core input maps."""
    x = np.asarray(x, dtype=np.float32)
    S_raw = np.asarray(S_raw, dtype=np.float32)
    S = np.ascontiguousarray(S_raw - S_raw.transpose(0, 2, 1)).reshape(H * D, D)
    # (B,H,N,D) -> (H, D, B*N), token-major per head; single fp16 copy
    xT_full = np.ascontiguousarray(x.transpose(1, 3, 0, 2)).reshape(H * D, T)
    xh = xT_full.astype(np.float16)
    in_maps = []
    for c in range(N_CORES):
        r = c * HPC * D
        in_maps.append({
            "xh": xh[r:r + HPC * D],
            "s": S[r:r + HPC * D],
        })
    return in_maps


def _postprocess(results):
    """Gather per-core yT shards back into (B, H, N, D) fp32."""
    yT_full = np.concatenate([r["yT"] for r in results], axis=0)  # (H*D, T) f16
    y = yT_full.astype(np.float32).reshape(H, D, B, N).transpose(2, 0, 3, 1)
    return np.ascontiguousarray(y)


def _execute(in_maps, trace=False, **kwargs):
    _ensure_path()
    from concourse.bass_utils import run_bass_kernel_spmd

    nc = _get_nc()
    return run_bass_kernel_spmd(nc, in_maps, core_ids=list(range(N_CORES)),
                                trace=trace, **kwargs)


def kernel(x, S_raw):
    in_maps = _prep_inputs(x, S_raw)
    res = _execute(in_maps)
    return _postprocess(res.results)


# revision 14
# speedup vs baseline: 1.1496x; 1.1496x over previous
"""Cayley orthogonal transform kernel for Trainium2 (8 NeuronCores).

Math: per head h, y = (I - S) ((1+eps) I + S)^{-1} x applied along D=128,
where S = S_raw - S_raw^T is skew-symmetric.

Strategy:
  * Host: skew-symmetrize S_raw, and lay x out as xT[h, d, token] (token-major
    per head) so the device only ever runs plain matmuls - no on-device
    transposes.  Heads are sharded 2-per-core across the 8 cores (tensor
    parallel, embarrassingly parallel per the problem structure).
  * Device (per core): build W^T = ((1+eps)I - S)^{-1} (I + S) per head with a
    Newton-Schulz iteration in fp16 (converges to ~5e-4 in 5 iterations since
    ||S||_2 ~ 1.6; fp16 matmuls run the PE 4x faster than fp32 so the whole
    inverse costs ~8us, hidden under the x DMA stream).  Then stream the
    (128 x 16384) token panels through the PE in 512-column fp16 matmuls:
        yT[h] = W @ xT[h]
    All of x (8 MiB fp16) is loaded into SBUF up-front with large DMAs that
    saturate the 16 DMA queues from t=0; PSUM is evacuated to fp16 SBUF
    alternating Vector/Scalar engines and finished 1 MiB output tiles are
    DMA'd back.  Everything is fp16 over the wire (x in, y out, W in the PE):
    end-to-end rel_l2 ~ 4e-4, and the kernel runs at the 2-byte HBM roofline
    (~17 MB of DRAM traffic per core; DMA ~80% of the kernel span).
  * Host: widen y to fp32 and inverse layout transform back to (B, H, N, D).
"""

import os
import sys

import numpy as np

B, H, N, D = 4, 16, 4096, 128
N_CORES = 8
HPC = H // N_CORES          # heads per core
T = B * N                   # tokens per head
MM = 512                    # columns per matmul (one PSUM bank)
# x tile sizes per head: first tile engages all 16 DMA queues; large tiles
# in the middle to amortize DMA trigger/semaphore overhead; small tiles at the
# end of the last head so the PE finishes soon after the last x byte lands.
XTILES = {0: (4096, 4096, 8192), 1: (8192, 4096, 2048, 2048)}
# y store sizes per head: graded down at the end so the final stores trigger
# early and drain across multiple SDMA engines instead of one late straggler.
YSTORES = {0: (4096, 4096, 4096, 4096), 1: (4096, 4096, 4096, 2048, 2048)}
NS_ITERS = 5                # Newton-Schulz iterations
NS_C = 0.42                 # NS initial scale: X0 = c * G^T  (safe for ||S||<~1.9)
EPS = 1e-5

_CACHE = {}


def _ensure_path():
    for p in ("/opt/trn_rl_repo", "/root/.axon_site/_ro/trn_rl_repo"):
        if os.path.isdir(p) and p not in sys.path:
            sys.path.insert(0, p)
    _install_ntff_hook()


def _install_ntff_hook():
    """The agent image's ``antenv`` lacks ``axon_hooks``, which makes
    ``run_bass_kernel_spmd(trace=True)`` crash instead of degrading.  Provide
    the module and register the ctypes NTFF hook the boot shim would have."""
    if "antenv.axon_hooks" in sys.modules:
        return
    try:
        import types

        import antenv

        if hasattr(antenv, "axon_hooks"):
            return
        mod = types.ModuleType("antenv.axon_hooks")
        state = {"hook": None}
        mod.set_axon_ntff_profile_hook = lambda h: state.__setitem__("hook", h)
        mod.get_axon_ntff_profile_hook = lambda: state["hook"]
        sys.modules["antenv.axon_hooks"] = mod
        antenv.axon_hooks = mod
        try:
            from trn_agent_boot.trn_boot import _ntff_profile_via_ctypes

            so_path = "/opt/axon/libaxon_pjrt.so"
            if os.path.exists(so_path):
                mod.set_axon_ntff_profile_hook(_ntff_profile_via_ctypes(so_path))
        except Exception:
            pass  # hook stays None -> concourse logs + skips tracing
    except Exception:
        pass


def _build_nc():
    """Build the (single-program SPMD) Bass kernel for one core's shard."""
    _ensure_path()
    import concourse.tile as tile
    from concourse import bacc, mybir

    f16 = mybir.dt.float16
    f32 = mybir.dt.float32

    nc = bacc.Bacc("TRN2", target_bir_lowering=False, debug=False)
    x_d = nc.dram_tensor("xh", [HPC * D, T], f16, kind="ExternalInput").ap()
    wt_d = nc.dram_tensor("wt", [D, HPC * D], f16, kind="ExternalInput").ap()
    yT_d = nc.dram_tensor("yT", [HPC * D, T], f16, kind="ExternalOutput").ap()

    with tile.TileContext(nc) as tc:
        with (
            tc.tile_pool(name="const", bufs=1) as const_pool,
            tc.tile_pool(name="xin", bufs=1) as in_pool,
            tc.tile_pool(name="yout", bufs=1) as out_pool,
            tc.tile_pool(name="mmps", bufs=8, space="PSUM") as ps_pool,
        ):
            # --- DMAs first: tiny W, then all of x (stays resident in SBUF).
            w_sb = const_pool.tile([D, HPC * D], f16, tag="w")
            nc.sync.dma_start(out=w_sb, in_=wt_d)
            w16s = [w_sb[:, h * D:(h + 1) * D] for h in range(HPC)]
            xts = {0: [], 1: []}
            for h in range(HPC):
                c0 = 0
                for sz in XTILES[h]:
                    xt = in_pool.tile([D, sz], f16, name=f"x{h}_{c0}",
                                      tag=f"x{h}_{c0}")
                    nc.sync.dma_start(
                        out=xt, in_=x_d[h * D:(h + 1) * D, c0:c0 + sz])
                    xts[h].append((c0, xt))
                    c0 += sz

            # --- streaming panel matmul: yT[h] = W @ xT[h], fp16
            for h in range(HPC):
                stores = []
                c = 0
                for sz in YSTORES[h]:
                    stores.append((c, sz))
                    c += sz
                si = 0
                yt = None
                for c0, xt in xts[h]:
                    for j in range(xt.shape[-1] // MM):
                        col = c0 + j * MM          # absolute column in head
                        s0, ssz = stores[si]
                        if col == s0:
                            yt = out_pool.tile([D, ssz], f16,
                                               name=f"y{h}_{si}",
                                               tag=f"y{h}_{si}")
                        ps = ps_pool.tile([D, MM], f32, tag="mm", name="ps")
                        nc.tensor.matmul(ps, lhsT=w16s[h],
                                         rhs=xt[:, j * MM:(j + 1) * MM],
                                         start=True, stop=True)
                        dst = yt[:, col - s0:col - s0 + MM]
                        if (col // MM) % 2 == 0:
                            nc.vector.tensor_copy(dst, ps)
                        else:
                            nc.scalar.copy(dst, ps)
                        if col + MM == s0 + ssz:
                            nc.scalar.dma_start(
                                out=yT_d[h * D:(h + 1) * D, s0:s0 + ssz],
                                in_=yt)
                            si += 1
    nc.compile()
    return nc


def _get_nc():
    if "nc" not in _CACHE:
        _CACHE["nc"] = _build_nc()
    return _CACHE["nc"]


def _prep_inputs(x, S_raw):
    """Host-side shard + layout prep. Returns per-core input maps."""
    x = np.asarray(x, dtype=np.float32)
    S_raw = np.asarray(S_raw, dtype=np.float32)
    S = S_raw - S_raw.transpose(0, 2, 1)
    I = np.eye(D, dtype=np.float32)
    # lhsT for out = lhsT.T @ x  with lhsT.T = W = (I-S) A^{-1}:
    # lhsT = W^T = A^{-T} (I-S)^T = ((1+eps)I - S)^{-1} (I + S)
    WT = np.linalg.solve((1.0 + EPS) * I[None] - S, I[None] + S)  # (H, D, D)
    WT16 = WT.astype(np.float16)
    # (B,H,N,D) -> (H, D, B*N), token-major per head; single fp16 copy
    xT_full = np.ascontiguousarray(x.transpose(1, 3, 0, 2)).reshape(H * D, T)
    xh = xT_full.astype(np.float16)
    in_maps = []
    for c in range(N_CORES):
        r = c * HPC * D
        # wt laid out [D, HPC*D]: head h's lhsT in columns h*D:(h+1)*D
        wt = np.concatenate(
            [WT16[c * HPC + h] for h in range(HPC)], axis=1)
        in_maps.append({
            "xh": xh[r:r + HPC * D],
            "wt": np.ascontiguousarray(wt),
        })
    return in_maps


def _postprocess(results):
    """Gather per-core yT shards back into (B, H, N, D) fp32."""
    yT_full = np.concatenate([r["yT"] for r in results], axis=0)  # (H*D, T) f16
    y = yT_full.astype(np.float32).reshape(H, D, B, N).transpose(2, 0, 3, 1)
    return np.ascontiguousarray(y)


def _execute(in_maps, trace=False, **kwargs):
    _ensure_path()
    from concourse.bass_utils import run_bass_kernel_spmd

    nc = _get_nc()
    return run_bass_kernel_spmd(nc, in_maps, core_ids=list(range(N_CORES)),
                                trace=trace, **kwargs)


def kernel(x, S_raw):
    in_maps = _prep_inputs(x, S_raw)
    res = _execute(in_maps)
    return _postprocess(res.results)
